# revision 41
# baseline (speedup 1.0000x reference)
"""Trainium2 Bass kernel for DeepTemplateMatchingModule (see header notes).

Sharding: core c owns eval+template samples {2c, 2c+1} (data parallel), the
GRU cross-sample hidden chain is broken every L=16 positions and re-warmed
with W=32 steps (error ~1e-9 relative by contraction; validated end-to-end
rel err ~8e-4 including bf16 pool/lin1).  conv1..conv3 compose into a single
1->64 13x13 conv (no nonlinearity between them): 6x fewer MACs.
"""

import numpy as np
import ml_dtypes

bf16 = ml_dtypes.bfloat16

B, T, S, H = 16, 512, 496, 64
L = 8           # GRU chain length
W = 16          # warmup steps
CH = 124        # parallel chains per branch (2 samples x 62)
GL = L * 128    # gi cols per branch: t(8) x (2 margin + 124 chains, pad 128)
MGP = 2480      # margin pooled extent: 5 channels (59..63) x 496

_CACHE = {}


def _compose_convs(w1, b1, w2, b2, w3, b3):
    def compose(wa, ba, wb, bb):
        O2, M, k2, _ = wb.shape
        _, I, k1, _ = wa.shape
        kc = k1 + k2 - 1
        wcm = np.zeros((O2, I, kc, kc), np.float64)
        wa64 = wa.astype(np.float64)
        wb64 = wb.astype(np.float64)
        for p in range(k2):
            for q in range(k2):
                wcm[:, :, p:p + k1, q:q + k1] += np.einsum(
                    'om,mikl->oikl', wb64[:, :, p, q], wa64)
        bcm = wb64.sum((2, 3)) @ ba.astype(np.float64) + bb
        return wcm, bcm

    wc12, bc12 = compose(w1, b1, w2, b2)
    wc, bc = compose(wc12, bc12, w3, b3)
    return wc[:, 0].astype(np.float32), bc.astype(np.float32)  # (64,13,13),(64,)


def _host_prep(inputs):
    wc, bc = _compose_convs(inputs['conv1_w'], inputs['conv1_b'],
                            inputs['conv2_w'], inputs['conv2_b'],
                            inputs['conv3_w'], inputs['conv3_b'])

    kh_i, dw_i = np.meshgrid(np.arange(13), np.arange(16), indexing='ij')

    def conv_lhsT(side):
        t1 = np.zeros((128, 128), np.float32)
        t2 = np.zeros((81, 128), np.float32)
        for jj in range(2):
            j = 2 * jj + side
            kw = dw_i - j
            ok = (kw >= 0) & (kw <= 12)
            for co in range(64):
                col = jj * 64 + co
                v = np.where(ok, wc[co][kh_i, np.clip(kw, 0, 12)], 0.0)
                t1[:, col] = v[:8].reshape(-1)
                t2[1:81, col] = v[8:].reshape(-1)
                t2[0, col] = bc[co]
        return t1, t2

    convA1, convA2 = conv_lhsT(0)
    convB1, convB2 = conv_lhsT(1)


    L1 = inputs['lin1_w']
    lin1w = np.zeros((58, 64 * 64), np.float32)
    for cp in range(64):
        lin1w[:, cp * 64:(cp + 1) * 64] = L1[:, cp * 58:(cp + 1) * 58].T
    lin1w = lin1w.astype(bf16)
    lin1b = inputs['lin1_b'].reshape(1, 64).astype(bf16)

    W_ih, b_ih = inputs['W_ih'], inputs['b_ih']
    W_hh, b_hh = inputs['W_hh'], inputs['b_hh']
    consts = dict(
        convA1=convA1, convA2=convA2, convB1=convB1, convB2=convB2,
        lin1w=lin1w, lin1b=lin1b,
        giW_rz=np.ascontiguousarray(W_ih[:128].T).astype(bf16),
        giW_n=np.ascontiguousarray(W_ih[128:].T).astype(bf16),
        giB_rz=b_ih[:128].reshape(1, 128).astype(bf16),
        giB_n=b_ih[128:].reshape(1, 64).astype(bf16),
        gruW_rz=np.concatenate(
            [np.concatenate([W_hh[:64].T, W_hh[64:128].T], 1),
             b_hh[:128].reshape(1, 128)], 0).astype(bf16),
        gruW_n=np.concatenate([W_hh[128:].T, b_hh[128:].reshape(1, 64)], 0).astype(bf16),
        attw=np.ascontiguousarray(inputs['att_w'].reshape(1, 64).T).astype(bf16),
        lin3T=np.ascontiguousarray(inputs['lin3_w'].T).astype(np.float32),
        lin3b=inputs['lin3_b'].reshape(1, 128).astype(np.float32),
        clsT=np.ascontiguousarray(inputs['cls_w'].T).astype(np.float32),
        clsb=inputs['cls_b'].reshape(1, 2).astype(np.float32),
    )

    for name in ['convA1', 'convA2', 'convB1', 'convB2']:
        consts[name] = consts[name].astype(bf16)

    rows16 = 4 * np.arange(29)[:, None] + np.arange(16)[None, :]  # (29,16)

    def build_slabs(x_u):
        # im2col slabs pre-expanded on host: s1[k, kh*16+dw, h] = x[4k+dw, kh+h]
        A = np.stack([x_u[:, kh:kh + 500] for kh in range(13)])   # (13,128,500)
        s1 = A[np.arange(8)[None, :, None], rows16[:, None, :], :]
        s1 = s1.reshape(29, 128, 500)
        s2 = np.empty((29, 81, 500), x_u.dtype)
        s2[:, 0] = 1.0
        s2[:, 1:] = A[8 + np.arange(5)[None, :, None],
                      rows16[:, None, :], :].reshape(29, 80, 500)
        return s1, s2

    ev, tm = inputs['evaluation'], inputs['template']
    in_maps = []
    for c in range(8):
        x6 = np.zeros((6, 128, 512), np.float32)
        if c > 0:
            x6[0] = ev[2 * c - 1]
            x6[3] = tm[2 * c - 1]
        x6[1], x6[2] = ev[2 * c], ev[2 * c + 1]
        x6[4], x6[5] = tm[2 * c], tm[2 * c + 1]
        x6 = x6.astype(bf16)
        s1s = np.empty((6, 29, 128, 500), bf16)
        s2s = np.empty((6, 29, 81, 500), bf16)
        for u in range(6):
            s1s[u], s2s[u] = build_slabs(x6[u])
        kill = np.ones((64, 2 * CH), np.float32)
        if c == 0:
            kill[:, 0] = 0.0
            kill[:, CH] = 0.0
        m = dict(consts)
        m['s1s'] = s1s
        m['s2s'] = s2s
        m['kill'] = kill
        in_maps.append(m)
    return in_maps


def _ap_mod(ap, dims, extra_offset=0):
    """Rebuild an AP keeping its partition dim, custom free dims, offset shift."""
    import dataclasses
    d0 = [ap.ap[0][0], ap.ap[0][1]]
    return dataclasses.replace(ap, ap=[d0] + [list(d) for d in dims],
                               offset=ap.offset + extra_offset)


def _ap_raw(ap, dims, extra_offset=0):
    """Rebuild an AP with fully custom dims (DRAM / linear)."""
    import dataclasses
    return dataclasses.replace(ap, ap=[list(d) for d in dims],
                               offset=ap.offset + extra_offset)



# ---------------------------------------------------------------------------
# Walrus workaround: this toolchain's codegen accepts at most ONE sem-wait per
# instruction ("Too many sync wait commands"), but Tile emits several.  Split
# every instruction with N>1 waits into N-1 preceding same-engine NoOps
# carrying one wait each, applied to the BIR json just before compile.
def _split_waits_bir(bir_bytes):
    import orjson
    m = orjson.loads(bir_bytes)
    ctr = [0]
    for fn in m['functions']:
        for bb in fn.get('blocks') or []:
            insts = bb.get('instructions')
            if not insts:
                continue
            out = []
            for inst in insts:
                si = inst.get('sync_info')
                waits = (si or {}).get('on_wait') or []
                if len(waits) > 1:
                    for w in waits[:-1]:
                        ctr[0] += 1
                        out.append({
                            'name': "%s_sw%d" % (inst['name'], ctr[0]),
                            'opcode': 'NoOp',
                            'engine': inst['engine'],
                            'ins': [], 'outs': [],
                            'debug': inst.get('debug'),
                            'sync_info': {'on_update': [], 'on_wait': [w]},
                        })
                    si['on_wait'] = [waits[-1]]
                out.append(inst)
            bb['instructions'] = out
    return orjson.dumps(m)


def _install_bir_fix():
    if _CACHE.get('bir_fix'):
        return
    _CACHE['bir_fix'] = True
    import concourse.bass2jax as b2j
    import concourse.bass_utils as bu
    orig = bu.compile_bir_kernel

    def wrapped(bir_json, tmpdir, neff_name='file.neff'):
        if isinstance(bir_json, str):
            bir_json = bir_json.encode()
        return orig(_split_waits_bir(bir_json), tmpdir, neff_name=neff_name)

    b2j.compile_bir_kernel = wrapped
    bu.compile_bir_kernel = wrapped


def _build_program():
    import concourse.bass as bass
    import concourse.mybir as mybir
    import concourse.tile as tile
    from concourse.masks import make_identity

    f32 = mybir.dt.float32
    bft = mybir.dt.bfloat16
    AF = mybir.ActivationFunctionType
    ALU = mybir.AluOpType

    nc = bass.Bass()

    din = {}
    for name, shape, dt in [
        ('s1s', (6, 29, 128, 500), bft), ('s2s', (6, 29, 81, 500), bft),
        ('kill', (64, 2 * CH), f32),
        ('convA1', (128, 128), bft), ('convA2', (81, 128), bft),
        ('convB1', (128, 128), bft), ('convB2', (81, 128), bft),
        ('lin1w', (58, 4096), bft), ('lin1b', (1, 64), bft),
        ('giW_rz', (64, 128), bft), ('giW_n', (64, 64), bft),
        ('giB_rz', (1, 128), bft), ('giB_n', (1, 64), bft),
        ('gruW_rz', (65, 128), bft), ('gruW_n', (65, 64), bft),
        ('attw', (64, 1), bft), ('lin3T', (64, 128), f32),
        ('lin3b', (1, 128), f32), ('clsT', (128, 2), f32),
        ('clsb', (1, 2), f32),
    ]:
        din[name] = nc.declare_dram_parameter(name, list(shape), dt, isOutput=False)
    dout = nc.declare_dram_parameter('outloc', [2, 2], f32, isOutput=True)

    with tile.TileContext(nc) as tc:
        with tc.tile_pool(name='persist', bufs=1) as pp:
            P2gA = pp.tile([58, 31744], bft)
            P2gB = pp.tile([58, 31744], bft)
            PmA = pp.tile([58, MGP], bft)
            PmB = pp.tile([58, MGP], bft)
            gi_rz = pp.tile([128, 2 * GL], bft)
            gi_n = pp.tile([64, 2 * GL], bft)
            ETm = pp.tile([65, 2048], bft)
            hA = pp.tile([65, 2 * CH], bft)
            hB = pp.tile([65, 2 * CH], bft)
            ident = pp.tile([128, 128], bft)
            ones = pp.tile([1, 512], f32)
            onesb = pp.tile([1, 512], bft)

            cst = {}
            for name in ['convA1', 'convA2', 'convB1', 'convB2',
                         'lin1w', 'lin1b', 'giW_rz', 'giW_n',
                         'giB_rz', 'giB_n', 'gruW_rz',
                         'gruW_n', 'attw', 'lin3T', 'lin3b', 'clsT', 'clsb',
                         'kill']:
                t = pp.tile(list(din[name].shape), din[name].dtype, name=f'c_{name}')
                nc.sync.dma_start(t, din[name][:, :])
                cst[name] = t

            make_identity(nc, ident)
            nc.vector.memset(ones, 1.0)
            nc.vector.memset(onesb, 1.0)
            nc.vector.memset(ETm[64:65, :], 1.0)
            nc.vector.memset(hA, 0.0)
            nc.vector.memset(hA[64:65, :], 1.0)
            nc.vector.memset(hB[64:65, :], 1.0)

            # ================= PHASE A =================
            with tc.tile_pool(name='pA', bufs=2) as pa, \
                 tc.tile_pool(name='pAp', bufs=8, space='PSUM') as pap:

                def conv_unit(u, margin, P2g, Pm):
                    xs1 = din['s1s'][u]
                    xs2 = din['s2s'][u]
                    for b0 in range(0, 29, 8):
                        nb = min(8, 29 - b0)
                        W0 = nb * 500
                        s1x = pa.tile([128, W0], bft, tag='slab1',
                                      padded_shape=[128, 4000])
                        s2x = pa.tile([81, W0], bft, tag='slab2',
                                      padded_shape=[128, 4000])
                        # dst free = (ks, h); src (p, ks, h)
                        nc.sync.dma_start(
                            s1x[:, :],
                            _ap_raw(xs1[b0], [[500, 128], [64000, nb], [1, 500]]))
                        nc.sync.dma_start(
                            s2x[0:81, :],
                            _ap_raw(xs2[b0], [[500, 81], [40500, nb], [1, 500]]))

                        for ks in range(nb):
                            k = b0 + ks
                            sl = slice(ks * 500, ks * 500 + 500)
                            s1 = s1x[:, sl]
                            s2 = s2x[0:81, sl]
                            psA = pap.tile([128, 500], f32, tag='ps',
                                           padded_shape=[128, 512])
                            psB = pap.tile([128, 500], f32, tag='ps',
                                           padded_shape=[128, 512])
                            nc.tensor.matmul(psA, cst['convA1'], s1, start=True, stop=False)
                            nc.tensor.matmul(psA, cst['convA2'], s2, start=False, stop=True)
                            nc.tensor.matmul(psB, cst['convB1'], s1, start=True, stop=False)
                            nc.tensor.matmul(psB, cst['convB2'], s2, start=False, stop=True)
                            pwb = pa.tile([128, 500], bft, tag='pwb')
                            nc.scalar.activation(pwb, psB, AF.Copy)
                            pw = pa.tile([128, 500], bft, tag='pw')
                            nc.vector.tensor_tensor(pw, psA, pwb, op=ALU.max)
                            m2 = pa.tile([128, 499], bft, tag='m2', padded_shape=[128, 512])
                            nc.vector.tensor_tensor(m2, pw[:, 0:499], pw[:, 1:500], op=ALU.max)
                            m4 = pa.tile([128, 497], bft, tag='m4', padded_shape=[128, 512])
                            nc.vector.tensor_tensor(m4, m2[:, 0:497], m2[:, 2:499], op=ALU.max)
                            pooled = pa.tile([128, 496], bft, tag='pool')
                            nc.vector.tensor_tensor(pooled, m4[:, 0:496], pw[:, 4:500], op=ALU.max)
                            wrow = 2 * k
                            if margin:
                                # only channels 59..63 feed warmup gi; store
                                # them in full: Pm[w, co''*496+s], co''=co-59
                                for jj in range(2):
                                    nc.scalar.dma_start(
                                        _ap_mod(Pm[wrow + jj:wrow + jj + 1, 0:1],
                                                [[496, 5], [1, 496]]),
                                        pooled[jj * 64 + 59:jj * 64 + 64, 0:496])
                            else:
                                nc.scalar.dma_start(
                                    _ap_mod(P2g[wrow:wrow + 2, 0:1],
                                            [[496, 64], [1, 496]]),
                                    pooled[0:128, 0:496])

                def lin_gi(br, b_loc, margin, src):
                    n = W if margin else 496
                    pl = pap.tile([64, 512], f32, tag='ps',
                                  padded_shape=[128, 512], name='pl')[:, 0:n]
                    for cp in range(64):
                        rhs = _ap_mod(src[0:58, 0:1], [[64, n]],
                                      (1456 if margin else 0) + cp)
                        nc.tensor.matmul(pl, cst['lin1w'][:, cp * 64:(cp + 1) * 64],
                                         rhs, start=(cp == 0), stop=False)
                    nc.tensor.matmul(pl, cst['lin1b'], onesb[0:1, 0:n],
                                     start=False, stop=True)
                    lo = pa.tile([64, 512], bft, tag='lo',
                                 padded_shape=[128, 512], name='lo')[:, 0:n]
                    nc.scalar.activation(lo, pl, AF.Copy)
                    gparts = [('giW_rz', 'giB_rz', gi_rz, 128),
                              ('giW_n', 'giB_n', gi_n, 64)]
                    for wname, bname, store, gp in gparts:
                        pg = pap.tile([gp, 512], f32, tag='ps',
                                      padded_shape=[128, 512],
                                      name='pg_' + wname)[:, 0:n]
                        nc.tensor.matmul(pg, cst[wname], lo, start=True, stop=False)
                        nc.tensor.matmul(pg, cst[bname], onesb[0:1, 0:n], start=False, stop=True)
                        if margin:
                            # psum col i = (c2, t8): dst col = t*128 + c
                            nc.scalar.activation(
                                _ap_mod(store[0:gp, 0:1], [[1, 2], [128, L]], br * GL),
                                pg, AF.Copy)
                        else:
                            # psum col s = (j62, t8): dst col = t*128 + (62*b_loc + j + 2)
                            off = br * GL + 62 * b_loc + 2
                            nc.scalar.activation(
                                _ap_mod(store[0:gp, 0:1], [[1, 62], [128, L]], off),
                                _ap_mod(pg, [[L, 62], [1, L]]), AF.Copy)

                # software pipeline: lin_gi(u) issues after conv_unit(u+1) so
                # the PE never head-of-line blocks on u's P2g store DMAs.
                unit_bufs = [PmA, P2gA, P2gB, PmB, P2gA, P2gB]
                units = [(0, 0, True), (0, 0, False), (0, 1, False),
                         (1, 0, True), (1, 0, False), (1, 1, False)]
                pending = []
                for u, (br, b_loc, margin) in enumerate(units):
                    buf = unit_bufs[u]
                    if margin:
                        conv_unit(u, margin, None, buf)
                    else:
                        conv_unit(u, margin, buf, None)
                    if pending:
                        lin_gi(*pending.pop())
                    pending.append((br, b_loc, margin, buf))
                lin_gi(*pending.pop())

            # ================= PHASE B: GRU =================
            with tc.tile_pool(name='pB', bufs=2) as pb, \
                 tc.tile_pool(name='pBp', bufs=2, space='PSUM') as pbp:

                NC2 = 2 * CH

                def h_ap(t, p=65):
                    return _ap_mod(ETm[0:p, 0:1], [[1024, 2], [L, CH]], t)

                def gi_ap(store, p, t):
                    q, tp = divmod(t, L)
                    return _ap_mod(store[0:p, 0:1], [[GL, 2], [1, CH]],
                                   tp * 128 + 2 + q)

                killed = pp.tile([65, NC2], bft)
                nc.vector.memset(killed[64:65, :], 1.0)

                for i, t in enumerate(range(-W, L)):
                    if t < 0:
                        h_in = hA if i % 2 == 0 else hB
                        h_out_ap = (hB if i % 2 == 0 else hA)[0:64, :]
                    elif t == 0:
                        h_in = killed
                        h_out_ap = h_ap(0, 64)
                    else:
                        h_in = None
                        h_out_ap = h_ap(t, 64)

                    h_in_ap = h_in[0:65, :] if h_in is not None else h_ap(t - 1)
                    h_in64 = h_in[0:64, :] if h_in is not None else h_ap(t - 1, 64)

                    prz = pbp.tile([128, NC2], f32, tag='grz')
                    pn = pbp.tile([64, NC2], f32, tag='gn', padded_shape=[128, NC2])
                    nc.tensor.matmul(prz, cst['gruW_rz'], h_in_ap, start=True, stop=False)
                    nc.tensor.matmul(prz, ident, gi_ap(gi_rz, 128, t), start=False, stop=True)
                    nc.tensor.matmul(pn, cst['gruW_n'], h_in_ap, start=True, stop=True)
                    r = pb.tile([64, NC2], f32, tag='r', padded_shape=[128, NC2])
                    nc.scalar.activation(r, prz[0:64, :], AF.Sigmoid)
                    z = pb.tile([64, NC2], f32, tag='z', padded_shape=[128, NC2])
                    nc.scalar.activation(z, prz[64:128, :], AF.Sigmoid)
                    t2 = pb.tile([64, NC2], f32, tag='t2', padded_shape=[128, NC2])
                    nc.vector.tensor_mul(t2, r, pn)
                    npre = pb.tile([64, NC2], f32, tag='npre', padded_shape=[128, NC2])
                    nc.vector.tensor_add(npre, t2, gi_ap(gi_n, 64, t))
                    nt = pb.tile([64, NC2], f32, tag='nt', padded_shape=[128, NC2])
                    nc.scalar.activation(nt, npre, AF.Tanh)
                    dmn = pb.tile([64, NC2], f32, tag='dmn', padded_shape=[128, NC2])
                    nc.vector.scalar_tensor_tensor(dmn, nt, -1.0, h_in64,
                                                   op0=ALU.mult, op1=ALU.add)
                    e = pb.tile([64, NC2], f32, tag='e', padded_shape=[128, NC2])
                    nc.vector.tensor_mul(e, z, dmn)
                    nc.vector.tensor_add(h_out_ap, nt, e)
                    if t == -1:
                        last = hB if i % 2 == 0 else hA
                        nc.vector.tensor_mul(killed[0:64, :], last[0:64, :], cst['kill'])

            # ================= PHASE C =================
            with tc.tile_pool(name='pC', bufs=2) as pc_, \
                 tc.tile_pool(name='pCe', bufs=4) as pce, \
                 tc.tile_pool(name='pCp', bufs=2, space='PSUM') as pcp, \
                 tc.tile_pool(name='pCs', bufs=1, space='PSUM') as pcs:
                NB = [(0, 128), (128, 128), (256, 128), (384, 112)]
                for b_loc in range(2):
                    Es = ETm[0:64, b_loc * 496:b_loc * 496 + 496]
                    Ts = ETm[0:64, 1024 + b_loc * 496:1024 + b_loc * 496 + 496]
                    etiles, tmts = [], []
                    for (nb0, nbs) in NB:
                        psc = pcp.tile([128, 512], f32, tag='sc', name='psc')[0:nbs, 0:496]
                        nc.tensor.matmul(psc, Ts[:, nb0:nb0 + nbs], Es, start=True, stop=True)
                        nmx = pc_.tile([128, 1], f32, tag='nmx', name='nmx')[0:nbs, :]
                        nc.vector.tensor_reduce(nmx, psc, axis=mybir.AxisListType.X,
                                                op=ALU.max, negate=True)
                        et = pce.tile([128, 496], bft, tag='et', name='et')[0:nbs, :]
                        ssum = pc_.tile([128, 1], f32, tag='ssum', name='ssum')[0:nbs, :]
                        nc.scalar.activation(et, psc, AF.Exp, bias=nmx, accum_out=ssum)
                        rs = pc_.tile([128, 1], f32, tag='rs', name='rs')[0:nbs, :]
                        nc.vector.reciprocal(rs, ssum)
                        ptt = pcp.tile([128, 512], bft, tag='ptt', bufs=1, name='ptt')[0:nbs, 0:64]
                        nc.tensor.transpose(ptt, Ts[:, nb0:nb0 + nbs], ident[0:64, 0:64])
                        tmt = pce.tile([128, 64], bft, tag='tmt', name='tmt')[0:nbs, :]
                        nc.scalar.activation(tmt, ptt, AF.Copy, scale=rs)
                        etiles.append(et)
                        tmts.append(tmt)
                    ptp = pcs.tile([64, 512], f32, tag='tp', padded_shape=[128, 512], name='ptp')[:, 0:496]
                    for q, (nb0, nbs) in enumerate(NB):
                        nc.tensor.matmul(ptp, tmts[q], etiles[q],
                                         start=(q == 0), stop=(q == 3))
                    da = pc_.tile([64, 496], f32, tag='da', padded_shape=[128, 496])
                    nc.vector.tensor_sub(da, ptp, Es)
                    da2 = pc_.tile([64, 496], f32, tag='da2', padded_shape=[128, 496])
                    nc.scalar.activation(da2, da, AF.Abs)
                    patt = pcs.tile([1, 512], f32, tag='chain', padded_shape=[128, 512], bufs=2, name='patt')[:, 0:496]
                    nc.tensor.matmul(patt, cst['attw'], Es, start=True, stop=True)
                    anm = pc_.tile([1, 1], f32, tag='anm', padded_shape=[128, 1])
                    nc.vector.tensor_reduce(anm, patt, axis=mybir.AxisListType.X,
                                            op=ALU.max, negate=True)
                    ea = pc_.tile([1, 496], f32, tag='ea', padded_shape=[128, 496])
                    asum = pc_.tile([1, 1], f32, tag='asum', padded_shape=[128, 1])
                    nc.scalar.activation(ea, patt, AF.Exp, bias=anm, accum_out=asum)
                    ars = pc_.tile([1, 1], f32, tag='ars', padded_shape=[128, 1])
                    nc.vector.reciprocal(ars, asum)
                    pab = pcs.tile([64, 512], f32, tag='pab', padded_shape=[128, 512], name='pab')[:, 0:496]
                    nc.tensor.matmul(pab, ones[0:1, 0:64], ea, start=True, stop=True)
                    junk = pc_.tile([64, 496], f32, tag='junk', padded_shape=[128, 496])
                    nc.vector.tensor_mul(junk, da2, pab)
                    rep = pc_.tile([64, 1], f32, tag='rep', padded_shape=[128, 1])
                    nc.vector.tensor_reduce(rep, junk, axis=mybir.AxisListType.X,
                                            op=ALU.add)
                    prsb = pcs.tile([64, 1], f32, tag='chain', padded_shape=[128, 512], bufs=2)
                    nc.tensor.matmul(prsb, ones[0:1, 0:64], ars, start=True, stop=True)
                    rsb = pc_.tile([64, 1], f32, tag='rsb', padded_shape=[128, 1])
                    nc.scalar.activation(rsb, prsb, AF.Copy)
                    h1 = pc_.tile([64, 1], f32, tag='h1', padded_shape=[128, 1])
                    nc.scalar.activation(h1, rep, AF.Relu, scale=rsb)
                    ph2 = pcs.tile([128, 1], f32, tag='chain', padded_shape=[128, 512], bufs=2)
                    nc.tensor.matmul(ph2, cst['lin3T'], h1, start=True, stop=False)
                    nc.tensor.matmul(ph2, cst['lin3b'], ones[0:1, 0:1], start=False, stop=True)
                    h2 = pc_.tile([128, 1], f32, tag='h2')
                    nc.scalar.activation(h2, ph2, AF.Relu)
                    po = pcs.tile([2, 1], f32, tag='chain', padded_shape=[128, 512], bufs=2)
                    nc.tensor.matmul(po, cst['clsT'], h2, start=True, stop=False)
                    nc.tensor.matmul(po, cst['clsb'], ones[0:1, 0:1], start=False, stop=True)
                    osb = pc_.tile([2, 1], f32, tag='osb', padded_shape=[128, 1])
                    nc.scalar.activation(osb, po, AF.Copy)
                    nc.sync.dma_start(dout[b_loc:b_loc + 1, 0:2], osb)

    return nc


def kernel(**inputs):
    _install_bir_fix()
    inputs = {k: np.asarray(v) for k, v in inputs.items()}
    in_maps = _host_prep(inputs)
    if 'nc' not in _CACHE:
        _CACHE['nc'] = _build_program()
    nc = _CACHE['nc']
    from concourse.bass_utils import run_bass_kernel_spmd
    res = run_bass_kernel_spmd(nc, in_maps, core_ids=list(range(8)))
    out = np.zeros((16, 2), np.float32)
    for c in range(8):
        out[2 * c:2 * c + 2] = res.results[c]['outloc']
    return out



# revision 47
# speedup vs baseline: 1.0914x; 1.0914x over previous
"""Trainium2 Bass kernel for DeepTemplateMatchingModule (see header notes).

Sharding: core c owns eval+template samples {2c, 2c+1} (data parallel), the
GRU cross-sample hidden chain is broken every L=16 positions and re-warmed
with W=32 steps (error ~1e-9 relative by contraction; validated end-to-end
rel err ~8e-4 including bf16 pool/lin1).  conv1..conv3 compose into a single
1->64 13x13 conv (no nonlinearity between them): 6x fewer MACs.
"""

import numpy as np
import ml_dtypes

bf16 = ml_dtypes.bfloat16

B, T, S, H = 16, 512, 496, 64
L = 8           # GRU chain length
W = 16          # warmup steps
CH = 124        # parallel chains per branch (2 samples x 62)
GL = L * 128    # gi cols per branch: t(8) x (2 margin + 124 chains, pad 128)
MGP = 2480      # margin pooled extent: 5 channels (59..63) x 496

_CACHE = {}


def _compose_convs(w1, b1, w2, b2, w3, b3):
    def compose(wa, ba, wb, bb):
        O2, M, k2, _ = wb.shape
        _, I, k1, _ = wa.shape
        kc = k1 + k2 - 1
        wcm = np.zeros((O2, I, kc, kc), np.float64)
        wa64 = wa.astype(np.float64)
        wb64 = wb.astype(np.float64)
        for p in range(k2):
            for q in range(k2):
                wcm[:, :, p:p + k1, q:q + k1] += np.einsum(
                    'om,mikl->oikl', wb64[:, :, p, q], wa64)
        bcm = wb64.sum((2, 3)) @ ba.astype(np.float64) + bb
        return wcm, bcm

    wc12, bc12 = compose(w1, b1, w2, b2)
    wc, bc = compose(wc12, bc12, w3, b3)
    return wc[:, 0].astype(np.float32), bc.astype(np.float32)  # (64,13,13),(64,)


def _host_prep(inputs):
    wc, bc = _compose_convs(inputs['conv1_w'], inputs['conv1_b'],
                            inputs['conv2_w'], inputs['conv2_b'],
                            inputs['conv3_w'], inputs['conv3_b'])

    kh_i, dw_i = np.meshgrid(np.arange(13), np.arange(16), indexing='ij')

    def conv_lhsT(side):
        t1 = np.zeros((128, 128), np.float32)
        t2 = np.zeros((81, 128), np.float32)
        for jj in range(2):
            j = 2 * jj + side
            kw = dw_i - j
            ok = (kw >= 0) & (kw <= 12)
            for co in range(64):
                col = jj * 64 + co
                v = np.where(ok, wc[co][kh_i, np.clip(kw, 0, 12)], 0.0)
                t1[:, col] = v[:8].reshape(-1)
                t2[1:81, col] = v[8:].reshape(-1)
                t2[0, col] = bc[co]
        return t1, t2

    convA1, convA2 = conv_lhsT(0)
    convB1, convB2 = conv_lhsT(1)


    L1 = inputs['lin1_w']
    lin1w = np.zeros((58, 64 * 64), np.float32)
    for cp in range(64):
        lin1w[:, cp * 64:(cp + 1) * 64] = L1[:, cp * 58:(cp + 1) * 58].T
    lin1w = lin1w.astype(bf16)
    lin1b = inputs['lin1_b'].reshape(1, 64).astype(bf16)

    W_ih, b_ih = inputs['W_ih'], inputs['b_ih']
    W_hh, b_hh = inputs['W_hh'], inputs['b_hh']
    consts = dict(
        convA1=convA1, convA2=convA2, convB1=convB1, convB2=convB2,
        lin1w=lin1w, lin1b=lin1b,
        giW_rz=np.ascontiguousarray(W_ih[:128].T).astype(bf16),
        giW_n=np.ascontiguousarray(W_ih[128:].T).astype(bf16),
        giB_rz=b_ih[:128].reshape(1, 128).astype(bf16),
        giB_n=b_ih[128:].reshape(1, 64).astype(bf16),
        gruW_rz=np.concatenate(
            [np.concatenate([W_hh[:64].T, W_hh[64:128].T], 1),
             b_hh[:128].reshape(1, 128)], 0).astype(bf16),
        gruW_n=np.concatenate([W_hh[128:].T, b_hh[128:].reshape(1, 64)], 0).astype(bf16),
        attw=np.ascontiguousarray(inputs['att_w'].reshape(1, 64).T).astype(bf16),
        lin3T=np.ascontiguousarray(inputs['lin3_w'].T).astype(np.float32),
        lin3b=inputs['lin3_b'].reshape(1, 128).astype(np.float32),
        clsT=np.ascontiguousarray(inputs['cls_w'].T).astype(np.float32),
        clsb=inputs['cls_b'].reshape(1, 2).astype(np.float32),
    )

    for name in ['convA1', 'convA2', 'convB1', 'convB2']:
        consts[name] = consts[name].astype(bf16)

    rows16 = 4 * np.arange(29)[:, None] + np.arange(16)[None, :]  # (29,16)

    def build_slabs(x_u):
        # im2col slabs pre-expanded on host: s1[k, kh*16+dw, h] = x[4k+dw, kh+h]
        A = np.stack([x_u[:, kh:kh + 500] for kh in range(13)])   # (13,128,500)
        s1 = A[np.arange(8)[None, :, None], rows16[:, None, :], :]
        s1 = s1.reshape(29, 128, 500)
        s2 = np.empty((29, 81, 500), x_u.dtype)
        s2[:, 0] = 1.0
        s2[:, 1:] = A[8 + np.arange(5)[None, :, None],
                      rows16[:, None, :], :].reshape(29, 80, 500)
        return s1, s2

    ev, tm = inputs['evaluation'], inputs['template']
    in_maps = []
    for c in range(8):
        x6 = np.zeros((6, 128, 512), np.float32)
        if c > 0:
            x6[0] = ev[2 * c - 1]
            x6[3] = tm[2 * c - 1]
        x6[1], x6[2] = ev[2 * c], ev[2 * c + 1]
        x6[4], x6[5] = tm[2 * c], tm[2 * c + 1]
        x6 = x6.astype(bf16)
        s1s = np.empty((6, 29, 128, 500), bf16)
        s2s = np.empty((6, 29, 81, 500), bf16)
        for u in range(6):
            s1s[u], s2s[u] = build_slabs(x6[u])
        kill = np.ones((64, 2 * CH), np.float32)
        if c == 0:
            kill[:, 0] = 0.0
            kill[:, CH] = 0.0
        m = dict(consts)
        m['s1s'] = s1s
        m['s2s'] = s2s
        m['kill'] = kill
        in_maps.append(m)
    return in_maps


def _ap_mod(ap, dims, extra_offset=0):
    """Rebuild an AP keeping its partition dim, custom free dims, offset shift."""
    import dataclasses
    d0 = [ap.ap[0][0], ap.ap[0][1]]
    return dataclasses.replace(ap, ap=[d0] + [list(d) for d in dims],
                               offset=ap.offset + extra_offset)


def _ap_raw(ap, dims, extra_offset=0):
    """Rebuild an AP with fully custom dims (DRAM / linear)."""
    import dataclasses
    return dataclasses.replace(ap, ap=[list(d) for d in dims],
                               offset=ap.offset + extra_offset)



# ---------------------------------------------------------------------------
# Walrus workaround: this toolchain's codegen accepts at most ONE sem-wait per
# instruction ("Too many sync wait commands"), but Tile emits several.  Split
# every instruction with N>1 waits into N-1 preceding same-engine NoOps
# carrying one wait each, applied to the BIR json just before compile.
def _split_waits_bir(bir_bytes):
    import orjson
    m = orjson.loads(bir_bytes)
    ctr = [0]
    for fn in m['functions']:
        for bb in fn.get('blocks') or []:
            insts = bb.get('instructions')
            if not insts:
                continue
            out = []
            for inst in insts:
                si = inst.get('sync_info')
                waits = (si or {}).get('on_wait') or []
                if len(waits) > 1:
                    for w in waits[:-1]:
                        ctr[0] += 1
                        out.append({
                            'name': "%s_sw%d" % (inst['name'], ctr[0]),
                            'opcode': 'NoOp',
                            'engine': inst['engine'],
                            'ins': [], 'outs': [],
                            'debug': inst.get('debug'),
                            'sync_info': {'on_update': [], 'on_wait': [w]},
                        })
                    si['on_wait'] = [waits[-1]]
                out.append(inst)
            bb['instructions'] = out
    return orjson.dumps(m)


def _install_bir_fix():
    if _CACHE.get('bir_fix'):
        return
    _CACHE['bir_fix'] = True
    import concourse.bass2jax as b2j
    import concourse.bass_utils as bu
    orig = bu.compile_bir_kernel

    def wrapped(bir_json, tmpdir, neff_name='file.neff'):
        if isinstance(bir_json, str):
            bir_json = bir_json.encode()
        return orig(_split_waits_bir(bir_json), tmpdir, neff_name=neff_name)

    b2j.compile_bir_kernel = wrapped
    bu.compile_bir_kernel = wrapped


def _build_program():
    import concourse.bass as bass
    import concourse.mybir as mybir
    import concourse.tile as tile
    from concourse.masks import make_identity

    f32 = mybir.dt.float32
    bft = mybir.dt.bfloat16
    AF = mybir.ActivationFunctionType
    ALU = mybir.AluOpType

    nc = bass.Bass()

    din = {}
    for name, shape, dt in [
        ('s1s', (6, 29, 128, 500), bft), ('s2s', (6, 29, 81, 500), bft),
        ('kill', (64, 2 * CH), f32),
        ('convA1', (128, 128), bft), ('convA2', (81, 128), bft),
        ('convB1', (128, 128), bft), ('convB2', (81, 128), bft),
        ('lin1w', (58, 4096), bft), ('lin1b', (1, 64), bft),
        ('giW_rz', (64, 128), bft), ('giW_n', (64, 64), bft),
        ('giB_rz', (1, 128), bft), ('giB_n', (1, 64), bft),
        ('gruW_rz', (65, 128), bft), ('gruW_n', (65, 64), bft),
        ('attw', (64, 1), bft), ('lin3T', (64, 128), f32),
        ('lin3b', (1, 128), f32), ('clsT', (128, 2), f32),
        ('clsb', (1, 2), f32),
    ]:
        din[name] = nc.declare_dram_parameter(name, list(shape), dt, isOutput=False)
    dout = nc.declare_dram_parameter('outloc', [2, 2], f32, isOutput=True)

    with tile.TileContext(nc) as tc:
        with tc.tile_pool(name='persist', bufs=1) as pp:
            P2gA = pp.tile([58, 31744], bft)
            P2gB = pp.tile([58, 31744], bft)
            PmA = pp.tile([58, MGP], bft)
            PmB = pp.tile([58, MGP], bft)
            gi_rz = pp.tile([128, 2 * GL], bft)
            gi_n = pp.tile([64, 2 * GL], bft)
            ETm = pp.tile([65, 2048], bft)
            hA = pp.tile([65, 2 * CH], bft)
            hB = pp.tile([65, 2 * CH], bft)
            ident = pp.tile([128, 128], bft)
            ones = pp.tile([1, 512], f32)
            onesb = pp.tile([1, 512], bft)

            cst = {}
            for name in ['convA1', 'convA2', 'convB1', 'convB2',
                         'lin1w', 'lin1b', 'giW_rz', 'giW_n',
                         'giB_rz', 'giB_n', 'gruW_rz',
                         'gruW_n', 'attw', 'lin3T', 'lin3b', 'clsT', 'clsb',
                         'kill']:
                t = pp.tile(list(din[name].shape), din[name].dtype, name=f'c_{name}')
                nc.sync.dma_start(t, din[name][:, :])
                cst[name] = t

            make_identity(nc, ident)
            nc.vector.memset(ones, 1.0)
            nc.vector.memset(onesb, 1.0)
            nc.vector.memset(ETm[64:65, :], 1.0)
            nc.vector.memset(hA, 0.0)
            nc.vector.memset(hA[64:65, :], 1.0)
            nc.vector.memset(hB[64:65, :], 1.0)

            # ================= PHASE A =================
            with tc.tile_pool(name='pA', bufs=2) as pa, \
                 tc.tile_pool(name='pAp', bufs=8, space='PSUM') as pap:

                def conv_unit(u, margin, P2g, Pm):
                    xs1 = din['s1s'][u]
                    xs2 = din['s2s'][u]
                    bats = []
                    for b0 in range(0, 29, 8):
                        nb = min(8, 29 - b0)
                        s1x = pa.tile([128, nb * 500], bft, tag='slab1',
                                      padded_shape=[128, 4000], name='s1x')
                        s2x = pa.tile([81, nb * 500], bft, tag='slab2',
                                      padded_shape=[128, 4000], name='s2x')
                        bats.append((b0, nb, s1x, s2x))

                    def load(bat):
                        b0, nb, s1x, s2x = bat
                        # dst free = (ks, h); src (p, ks, h)
                        nc.sync.dma_start(
                            s1x[:, :],
                            _ap_raw(xs1[b0], [[500, 128], [64000, nb], [1, 500]]))
                        nc.sync.dma_start(
                            s2x[0:81, :],
                            _ap_raw(xs2[b0], [[500, 81], [40500, nb], [1, 500]]))

                    for bat in bats[:2]:
                        load(bat)
                    for bi, (b0, nb, s1x, s2x) in enumerate(bats):
                        for ks in range(nb):
                            k = b0 + ks
                            sl = slice(ks * 500, ks * 500 + 500)
                            s1 = s1x[:, sl]
                            s2 = s2x[0:81, sl]
                            psA = pap.tile([128, 500], f32, tag='ps',
                                           padded_shape=[128, 512])
                            psB = pap.tile([128, 500], f32, tag='ps',
                                           padded_shape=[128, 512])
                            nc.tensor.matmul(psA, cst['convA1'], s1, start=True, stop=False)
                            nc.tensor.matmul(psA, cst['convA2'], s2, start=False, stop=True)
                            nc.tensor.matmul(psB, cst['convB1'], s1, start=True, stop=False)
                            nc.tensor.matmul(psB, cst['convB2'], s2, start=False, stop=True)
                            pwb = pa.tile([128, 500], bft, tag='pwb')
                            nc.scalar.activation(pwb, psB, AF.Copy)
                            pw = pa.tile([128, 500], bft, tag='pw')
                            nc.vector.tensor_tensor(pw, psA, pwb, op=ALU.max)
                            m2 = pa.tile([128, 499], bft, tag='m2', padded_shape=[128, 512])
                            nc.vector.tensor_tensor(m2, pw[:, 0:499], pw[:, 1:500], op=ALU.max)
                            m4 = pa.tile([128, 497], bft, tag='m4', padded_shape=[128, 512])
                            nc.vector.tensor_tensor(m4, m2[:, 0:497], m2[:, 2:499], op=ALU.max)
                            pooled = pa.tile([128, 496], bft, tag='pool')
                            nc.vector.tensor_tensor(pooled, m4[:, 0:496], pw[:, 4:500], op=ALU.max)
                            wrow = 2 * k
                            if margin:
                                # only channels 59..63 feed warmup gi; store
                                # them in full: Pm[w, co''*496+s], co''=co-59
                                for jj in range(2):
                                    nc.sync.dma_start(
                                        _ap_mod(Pm[wrow + jj:wrow + jj + 1, 0:1],
                                                [[496, 5], [1, 496]]),
                                        pooled[jj * 64 + 59:jj * 64 + 64, 0:496])
                            else:
                                nc.sync.dma_start(
                                    _ap_mod(P2g[wrow:wrow + 2, 0:1],
                                            [[496, 64], [1, 496]]),
                                    pooled[0:128, 0:496])
                        if bi + 2 < len(bats):
                            load(bats[bi + 2])

                def lin_gi(br, b_loc, margin, src):
                    n = W if margin else 496
                    pl = pap.tile([64, 512], f32, tag='ps',
                                  padded_shape=[128, 512], name='pl')[:, 0:n]
                    for cp in range(64):
                        rhs = _ap_mod(src[0:58, 0:1], [[64, n]],
                                      (1456 if margin else 0) + cp)
                        nc.tensor.matmul(pl, cst['lin1w'][:, cp * 64:(cp + 1) * 64],
                                         rhs, start=(cp == 0), stop=False)
                    nc.tensor.matmul(pl, cst['lin1b'], onesb[0:1, 0:n],
                                     start=False, stop=True)
                    lo = pa.tile([64, 512], bft, tag='lo',
                                 padded_shape=[128, 512], name='lo')[:, 0:n]
                    nc.scalar.activation(lo, pl, AF.Copy)
                    gparts = [('giW_rz', 'giB_rz', gi_rz, 128),
                              ('giW_n', 'giB_n', gi_n, 64)]
                    for wname, bname, store, gp in gparts:
                        pg = pap.tile([gp, 512], f32, tag='ps',
                                      padded_shape=[128, 512],
                                      name='pg_' + wname)[:, 0:n]
                        nc.tensor.matmul(pg, cst[wname], lo, start=True, stop=False)
                        nc.tensor.matmul(pg, cst[bname], onesb[0:1, 0:n], start=False, stop=True)
                        if margin:
                            # psum col i = (c2, t8): dst col = t*128 + c
                            nc.scalar.activation(
                                _ap_mod(store[0:gp, 0:1], [[1, 2], [128, L]], br * GL),
                                pg, AF.Copy)
                        else:
                            # psum col s = (j62, t8): dst col = t*128 + (62*b_loc + j + 2)
                            off = br * GL + 62 * b_loc + 2
                            nc.scalar.activation(
                                _ap_mod(store[0:gp, 0:1], [[1, 62], [128, L]], off),
                                _ap_mod(pg, [[L, 62], [1, L]]), AF.Copy)

                # software pipeline: lin_gi(u) issues after conv_unit(u+1) so
                # the PE never head-of-line blocks on u's P2g store DMAs.
                unit_bufs = [PmA, P2gA, P2gB, PmB, P2gA, P2gB]
                units = [(0, 0, True), (0, 0, False), (0, 1, False),
                         (1, 0, True), (1, 0, False), (1, 1, False)]
                pending = []
                for u, (br, b_loc, margin) in enumerate(units):
                    buf = unit_bufs[u]
                    if margin:
                        conv_unit(u, margin, None, buf)
                    else:
                        conv_unit(u, margin, buf, None)
                    if pending:
                        lin_gi(*pending.pop())
                    pending.append((br, b_loc, margin, buf))
                lin_gi(*pending.pop())

            # ================= PHASE B: GRU =================
            with tc.tile_pool(name='pB', bufs=2) as pb, \
                 tc.tile_pool(name='pBp', bufs=2, space='PSUM') as pbp:

                NC2 = 2 * CH

                def h_ap(t, p=65):
                    return _ap_mod(ETm[0:p, 0:1], [[1024, 2], [L, CH]], t)

                def gi_ap(store, p, t):
                    q, tp = divmod(t, L)
                    return _ap_mod(store[0:p, 0:1], [[GL, 2], [1, CH]],
                                   tp * 128 + 2 + q)

                killed = pp.tile([65, NC2], bft)
                nc.vector.memset(killed[64:65, :], 1.0)

                for i, t in enumerate(range(-W, L)):
                    if t < 0:
                        h_in = hA if i % 2 == 0 else hB
                        h_out_ap = (hB if i % 2 == 0 else hA)[0:64, :]
                    elif t == 0:
                        h_in = killed
                        h_out_ap = h_ap(0, 64)
                    else:
                        h_in = None
                        h_out_ap = h_ap(t, 64)

                    h_in_ap = h_in[0:65, :] if h_in is not None else h_ap(t - 1)
                    h_in64 = h_in[0:64, :] if h_in is not None else h_ap(t - 1, 64)

                    prz = pbp.tile([128, NC2], f32, tag='grz')
                    pn = pbp.tile([64, NC2], f32, tag='gn', padded_shape=[128, NC2])
                    nc.tensor.matmul(prz, cst['gruW_rz'], h_in_ap, start=True, stop=False)
                    nc.tensor.matmul(prz, ident, gi_ap(gi_rz, 128, t), start=False, stop=True)
                    nc.tensor.matmul(pn, cst['gruW_n'], h_in_ap, start=True, stop=True)
                    r = pb.tile([64, NC2], f32, tag='r', padded_shape=[128, NC2])
                    nc.scalar.activation(r, prz[0:64, :], AF.Sigmoid)
                    z = pb.tile([64, NC2], f32, tag='z', padded_shape=[128, NC2])
                    nc.scalar.activation(z, prz[64:128, :], AF.Sigmoid)
                    t2 = pb.tile([64, NC2], f32, tag='t2', padded_shape=[128, NC2])
                    nc.vector.tensor_mul(t2, r, pn)
                    npre = pb.tile([64, NC2], f32, tag='npre', padded_shape=[128, NC2])
                    nc.vector.tensor_add(npre, t2, gi_ap(gi_n, 64, t))
                    nt = pb.tile([64, NC2], f32, tag='nt', padded_shape=[128, NC2])
                    nc.scalar.activation(nt, npre, AF.Tanh)
                    dmn = pb.tile([64, NC2], f32, tag='dmn', padded_shape=[128, NC2])
                    nc.vector.scalar_tensor_tensor(dmn, nt, -1.0, h_in64,
                                                   op0=ALU.mult, op1=ALU.add)
                    e = pb.tile([64, NC2], f32, tag='e', padded_shape=[128, NC2])
                    nc.vector.tensor_mul(e, z, dmn)
                    nc.vector.tensor_add(h_out_ap, nt, e)
                    if t == -1:
                        last = hB if i % 2 == 0 else hA
                        nc.vector.tensor_mul(killed[0:64, :], last[0:64, :], cst['kill'])

            # ================= PHASE C =================
            with tc.tile_pool(name='pC', bufs=2) as pc_, \
                 tc.tile_pool(name='pCe', bufs=4) as pce, \
                 tc.tile_pool(name='pCp', bufs=2, space='PSUM') as pcp, \
                 tc.tile_pool(name='pCs', bufs=1, space='PSUM') as pcs:
                NB = [(0, 128), (128, 128), (256, 128), (384, 112)]
                for b_loc in range(2):
                    Es = ETm[0:64, b_loc * 496:b_loc * 496 + 496]
                    Ts = ETm[0:64, 1024 + b_loc * 496:1024 + b_loc * 496 + 496]
                    etiles, tmts = [], []
                    for (nb0, nbs) in NB:
                        psc = pcp.tile([128, 512], f32, tag='sc', name='psc')[0:nbs, 0:496]
                        nc.tensor.matmul(psc, Ts[:, nb0:nb0 + nbs], Es, start=True, stop=True)
                        nmx = pc_.tile([128, 1], f32, tag='nmx', name='nmx')[0:nbs, :]
                        nc.vector.tensor_reduce(nmx, psc, axis=mybir.AxisListType.X,
                                                op=ALU.max, negate=True)
                        et = pce.tile([128, 496], bft, tag='et', name='et')[0:nbs, :]
                        ssum = pc_.tile([128, 1], f32, tag='ssum', name='ssum')[0:nbs, :]
                        nc.scalar.activation(et, psc, AF.Exp, bias=nmx, accum_out=ssum)
                        rs = pc_.tile([128, 1], f32, tag='rs', name='rs')[0:nbs, :]
                        nc.vector.reciprocal(rs, ssum)
                        ptt = pcp.tile([128, 512], bft, tag='ptt', bufs=1, name='ptt')[0:nbs, 0:64]
                        nc.tensor.transpose(ptt, Ts[:, nb0:nb0 + nbs], ident[0:64, 0:64])
                        tmt = pce.tile([128, 64], bft, tag='tmt', name='tmt')[0:nbs, :]
                        nc.scalar.activation(tmt, ptt, AF.Copy, scale=rs)
                        etiles.append(et)
                        tmts.append(tmt)
                    ptp = pcs.tile([64, 512], f32, tag='tp', padded_shape=[128, 512], name='ptp')[:, 0:496]
                    for q, (nb0, nbs) in enumerate(NB):
                        nc.tensor.matmul(ptp, tmts[q], etiles[q],
                                         start=(q == 0), stop=(q == 3))
                    da = pc_.tile([64, 496], f32, tag='da', padded_shape=[128, 496])
                    nc.vector.tensor_sub(da, ptp, Es)
                    da2 = pc_.tile([64, 496], f32, tag='da2', padded_shape=[128, 496])
                    nc.scalar.activation(da2, da, AF.Abs)
                    patt = pcs.tile([1, 512], f32, tag='chain', padded_shape=[128, 512], bufs=2, name='patt')[:, 0:496]
                    nc.tensor.matmul(patt, cst['attw'], Es, start=True, stop=True)
                    anm = pc_.tile([1, 1], f32, tag='anm', padded_shape=[128, 1])
                    nc.vector.tensor_reduce(anm, patt, axis=mybir.AxisListType.X,
                                            op=ALU.max, negate=True)
                    ea = pc_.tile([1, 496], f32, tag='ea', padded_shape=[128, 496])
                    asum = pc_.tile([1, 1], f32, tag='asum', padded_shape=[128, 1])
                    nc.scalar.activation(ea, patt, AF.Exp, bias=anm, accum_out=asum)
                    ars = pc_.tile([1, 1], f32, tag='ars', padded_shape=[128, 1])
                    nc.vector.reciprocal(ars, asum)
                    pab = pcs.tile([64, 512], f32, tag='pab', padded_shape=[128, 512], name='pab')[:, 0:496]
                    nc.tensor.matmul(pab, ones[0:1, 0:64], ea, start=True, stop=True)
                    junk = pc_.tile([64, 496], f32, tag='junk', padded_shape=[128, 496])
                    nc.vector.tensor_mul(junk, da2, pab)
                    rep = pc_.tile([64, 1], f32, tag='rep', padded_shape=[128, 1])
                    nc.vector.tensor_reduce(rep, junk, axis=mybir.AxisListType.X,
                                            op=ALU.add)
                    prsb = pcs.tile([64, 1], f32, tag='chain', padded_shape=[128, 512], bufs=2)
                    nc.tensor.matmul(prsb, ones[0:1, 0:64], ars, start=True, stop=True)
                    rsb = pc_.tile([64, 1], f32, tag='rsb', padded_shape=[128, 1])
                    nc.scalar.activation(rsb, prsb, AF.Copy)
                    h1 = pc_.tile([64, 1], f32, tag='h1', padded_shape=[128, 1])
                    nc.scalar.activation(h1, rep, AF.Relu, scale=rsb)
                    ph2 = pcs.tile([128, 1], f32, tag='chain', padded_shape=[128, 512], bufs=2)
                    nc.tensor.matmul(ph2, cst['lin3T'], h1, start=True, stop=False)
                    nc.tensor.matmul(ph2, cst['lin3b'], ones[0:1, 0:1], start=False, stop=True)
                    h2 = pc_.tile([128, 1], f32, tag='h2')
                    nc.scalar.activation(h2, ph2, AF.Relu)
                    po = pcs.tile([2, 1], f32, tag='chain', padded_shape=[128, 512], bufs=2)
                    nc.tensor.matmul(po, cst['clsT'], h2, start=True, stop=False)
                    nc.tensor.matmul(po, cst['clsb'], ones[0:1, 0:1], start=False, stop=True)
                    osb = pc_.tile([2, 1], f32, tag='osb', padded_shape=[128, 1])
                    nc.scalar.activation(osb, po, AF.Copy)
                    nc.sync.dma_start(dout[b_loc:b_loc + 1, 0:2], osb)

    return nc


def kernel(**inputs):
    _install_bir_fix()
    inputs = {k: np.asarray(v) for k, v in inputs.items()}
    in_maps = _host_prep(inputs)
    if 'nc' not in _CACHE:
        _CACHE['nc'] = _build_program()
    nc = _CACHE['nc']
    from concourse.bass_utils import run_bass_kernel_spmd
    res = run_bass_kernel_spmd(nc, in_maps, core_ids=list(range(8)))
    out = np.zeros((16, 2), np.float32)
    for c in range(8):
        out[2 * c:2 * c + 2] = res.results[c]['outloc']
    return out



# revision 59
# speedup vs baseline: 1.3956x; 1.2787x over previous
"""Trainium2 Bass kernel for DeepTemplateMatchingModule (see header notes).

Sharding: core c owns eval+template samples {2c, 2c+1} (data parallel), the
GRU cross-sample hidden chain is broken every L=16 positions and re-warmed
with W=32 steps (error ~1e-9 relative by contraction; validated end-to-end
rel err ~8e-4 including bf16 pool/lin1).  conv1..conv3 compose into a single
1->64 13x13 conv (no nonlinearity between them): 6x fewer MACs.
"""

import numpy as np
import ml_dtypes

bf16 = ml_dtypes.bfloat16

B, T, S, H = 16, 512, 496, 64
L = 8           # GRU chain length
W = 16          # warmup steps
CH = 124        # parallel chains per branch (2 samples x 62)
GL = L * 128    # gi cols per branch: t(8) x (2 margin + 124 chains, pad 128)
MGP = 2480      # margin pooled extent: 5 channels (59..63) x 496

_CACHE = {}


def _compose_convs(w1, b1, w2, b2, w3, b3):
    def compose(wa, ba, wb, bb):
        O2, M, k2, _ = wb.shape
        _, I, k1, _ = wa.shape
        kc = k1 + k2 - 1
        wcm = np.zeros((O2, I, kc, kc), np.float64)
        wa64 = wa.astype(np.float64)
        wb64 = wb.astype(np.float64)
        for p in range(k2):
            for q in range(k2):
                wcm[:, :, p:p + k1, q:q + k1] += np.einsum(
                    'om,mikl->oikl', wb64[:, :, p, q], wa64)
        bcm = wb64.sum((2, 3)) @ ba.astype(np.float64) + bb
        return wcm, bcm

    wc12, bc12 = compose(w1, b1, w2, b2)
    wc, bc = compose(wc12, bc12, w3, b3)
    return wc[:, 0].astype(np.float32), bc.astype(np.float32)  # (64,13,13),(64,)


def _host_prep(inputs):
    wc, bc = _compose_convs(inputs['conv1_w'], inputs['conv1_b'],
                            inputs['conv2_w'], inputs['conv2_b'],
                            inputs['conv3_w'], inputs['conv3_b'])

    kh_i, dw_i = np.meshgrid(np.arange(13), np.arange(16), indexing='ij')

    def conv_lhsT(side):
        t1 = np.zeros((128, 128), np.float32)
        t2 = np.zeros((81, 128), np.float32)
        for jj in range(2):
            j = 2 * jj + side
            kw = dw_i - j
            ok = (kw >= 0) & (kw <= 12)
            for co in range(64):
                col = jj * 64 + co
                v = np.where(ok, wc[co][kh_i, np.clip(kw, 0, 12)], 0.0)
                t1[:, col] = v[:8].reshape(-1)
                t2[1:81, col] = v[8:].reshape(-1)
                t2[0, col] = bc[co]
        return t1, t2

    convA1, convA2 = conv_lhsT(0)
    convB1, convB2 = conv_lhsT(1)


    L1 = inputs['lin1_w']
    lin1w = np.zeros((58, 64 * 64), np.float32)
    for cp in range(64):
        lin1w[:, cp * 64:(cp + 1) * 64] = L1[:, cp * 58:(cp + 1) * 58].T
    lin1w2 = np.zeros((128, 64 * 64), np.float32)
    lin1w2[0:58] = lin1w
    lin1w2[64:122] = lin1w
    lin1w = lin1w2.astype(bf16)
    lin1b = inputs['lin1_b'].reshape(1, 64).astype(bf16)

    W_ih, b_ih = inputs['W_ih'], inputs['b_ih']
    W_hh, b_hh = inputs['W_hh'], inputs['b_hh']
    consts = dict(
        convA1=convA1, convA2=convA2, convB1=convB1, convB2=convB2,
        lin1w=lin1w, lin1b=lin1b,
        giW_rz=np.ascontiguousarray(W_ih[:128].T).astype(bf16),
        giW_n=np.ascontiguousarray(W_ih[128:].T).astype(bf16),
        giB_rz=b_ih[:128].reshape(1, 128).astype(bf16),
        giB_n=b_ih[128:].reshape(1, 64).astype(bf16),
        gruW_rz=np.concatenate(
            [np.concatenate([W_hh[:64].T, W_hh[64:128].T], 1),
             b_hh[:128].reshape(1, 128)], 0).astype(bf16),
        gruW_n=np.concatenate([W_hh[128:].T, b_hh[128:].reshape(1, 64)], 0).astype(bf16),
        attw=np.ascontiguousarray(inputs['att_w'].reshape(1, 64).T).astype(bf16),
        lin3T=np.ascontiguousarray(inputs['lin3_w'].T).astype(np.float32),
        lin3b=inputs['lin3_b'].reshape(1, 128).astype(np.float32),
        clsT=np.ascontiguousarray(inputs['cls_w'].T).astype(np.float32),
        clsb=inputs['cls_b'].reshape(1, 2).astype(np.float32),
    )

    for name in ['convA1', 'convA2', 'convB1', 'convB2']:
        consts[name] = consts[name].astype(bf16)

    rows16 = 4 * np.arange(29)[:, None] + np.arange(16)[None, :]  # (29,16)

    def build_slabs(x_u):
        # im2col slabs pre-expanded on host: s1[k, kh*16+dw, h] = x[4k+dw, kh+h]
        A = np.stack([x_u[:, kh:kh + 500] for kh in range(13)])   # (13,128,500)
        s1 = A[np.arange(8)[None, :, None], rows16[:, None, :], :]
        s1 = s1.reshape(29, 128, 500)
        s2 = np.empty((29, 81, 500), x_u.dtype)
        s2[:, 0] = 1.0
        s2[:, 1:] = A[8 + np.arange(5)[None, :, None],
                      rows16[:, None, :], :].reshape(29, 80, 500)
        return s1, s2

    ev, tm = inputs['evaluation'], inputs['template']
    in_maps = []
    for c in range(8):
        x6 = np.zeros((6, 128, 512), np.float32)
        if c > 0:
            x6[0] = ev[2 * c - 1]
            x6[3] = tm[2 * c - 1]
        x6[1], x6[2] = ev[2 * c], ev[2 * c + 1]
        x6[4], x6[5] = tm[2 * c], tm[2 * c + 1]
        x6 = x6.astype(bf16)
        s1s = np.empty((6, 29, 128, 500), bf16)
        s2s = np.empty((6, 29, 81, 500), bf16)
        for u in range(6):
            s1s[u], s2s[u] = build_slabs(x6[u])
        kill = np.ones((64, 2 * CH), np.float32)
        if c == 0:
            kill[:, 0] = 0.0
            kill[:, CH] = 0.0
        m = dict(consts)
        m['s1s'] = s1s
        m['s2s'] = s2s
        m['kill'] = kill
        in_maps.append(m)
    return in_maps


def _ap_mod(ap, dims, extra_offset=0):
    """Rebuild an AP keeping its partition dim, custom free dims, offset shift."""
    import dataclasses
    d0 = [ap.ap[0][0], ap.ap[0][1]]
    return dataclasses.replace(ap, ap=[d0] + [list(d) for d in dims],
                               offset=ap.offset + extra_offset)


def _ap_raw(ap, dims, extra_offset=0):
    """Rebuild an AP with fully custom dims (DRAM / linear)."""
    import dataclasses
    return dataclasses.replace(ap, ap=[list(d) for d in dims],
                               offset=ap.offset + extra_offset)



# ---------------------------------------------------------------------------
# Walrus workaround: this toolchain's codegen accepts at most ONE sem-wait per
# instruction ("Too many sync wait commands"), but Tile emits several.  Split
# every instruction with N>1 waits into N-1 preceding same-engine NoOps
# carrying one wait each, applied to the BIR json just before compile.
def _split_waits_bir(bir_bytes):
    import orjson
    m = orjson.loads(bir_bytes)
    ctr = [0]
    for fn in m['functions']:
        for bb in fn.get('blocks') or []:
            insts = bb.get('instructions')
            if not insts:
                continue
            out = []
            for inst in insts:
                si = inst.get('sync_info')
                waits = (si or {}).get('on_wait') or []
                if len(waits) > 1:
                    for w in waits[:-1]:
                        ctr[0] += 1
                        out.append({
                            'name': "%s_sw%d" % (inst['name'], ctr[0]),
                            'opcode': 'NoOp',
                            'engine': inst['engine'],
                            'ins': [], 'outs': [],
                            'debug': inst.get('debug'),
                            'sync_info': {'on_update': [], 'on_wait': [w]},
                        })
                    si['on_wait'] = [waits[-1]]
                out.append(inst)
            bb['instructions'] = out
    return orjson.dumps(m)


def _install_bir_fix():
    if _CACHE.get('bir_fix'):
        return
    _CACHE['bir_fix'] = True
    import concourse.bass2jax as b2j
    import concourse.bass_utils as bu
    orig = bu.compile_bir_kernel

    def wrapped(bir_json, tmpdir, neff_name='file.neff'):
        if isinstance(bir_json, str):
            bir_json = bir_json.encode()
        return orig(_split_waits_bir(bir_json), tmpdir, neff_name=neff_name)

    b2j.compile_bir_kernel = wrapped
    bu.compile_bir_kernel = wrapped


def _build_program():
    import concourse.bass as bass
    import concourse.mybir as mybir
    import concourse.tile as tile
    from concourse.masks import make_identity

    f32 = mybir.dt.float32
    bft = mybir.dt.bfloat16
    AF = mybir.ActivationFunctionType
    ALU = mybir.AluOpType

    nc = bass.Bass()

    din = {}
    for name, shape, dt in [
        ('s1s', (6, 29, 128, 500), bft), ('s2s', (6, 29, 81, 500), bft),
        ('kill', (64, 2 * CH), f32),
        ('convA1', (128, 128), bft), ('convA2', (81, 128), bft),
        ('convB1', (128, 128), bft), ('convB2', (81, 128), bft),
        ('lin1w', (128, 4096), bft), ('lin1b', (1, 64), bft),
        ('giW_rz', (64, 128), bft), ('giW_n', (64, 64), bft),
        ('giB_rz', (1, 128), bft), ('giB_n', (1, 64), bft),
        ('gruW_rz', (65, 128), bft), ('gruW_n', (65, 64), bft),
        ('attw', (64, 1), bft), ('lin3T', (64, 128), f32),
        ('lin3b', (1, 128), f32), ('clsT', (128, 2), f32),
        ('clsb', (1, 2), f32),
    ]:
        din[name] = nc.declare_dram_parameter(name, list(shape), dt, isOutput=False)
    dout = nc.declare_dram_parameter('outloc', [2, 2], f32, isOutput=True)

    with tile.TileContext(nc) as tc:
        with tc.tile_pool(name='persist', bufs=1) as pp:
            # A/B double buffers packed on partition rows 0-57 / 64-121 of one
            # tile (matmul rhs base_partition must be 0 or 64)
            P2g2 = pp.tile([128, 31744], bft)
            Pm2 = pp.tile([128, MGP], bft)
            gi_rz = pp.tile([128, 2 * GL], bft)
            gi_n = pp.tile([64, 2 * GL], bft)
            ETm = pp.tile([65, 2048], bft)
            hA = pp.tile([65, 2 * CH], bft)
            hB = pp.tile([65, 2 * CH], bft)
            ident = pp.tile([128, 128], bft)
            ones = pp.tile([1, 512], f32)
            onesb = pp.tile([1, 512], bft)

            cst = {}
            for name in ['convA1', 'convA2', 'convB1', 'convB2',
                         'lin1w', 'lin1b', 'giW_rz', 'giW_n',
                         'giB_rz', 'giB_n', 'gruW_rz',
                         'gruW_n', 'attw', 'lin3T', 'lin3b', 'clsT', 'clsb',
                         'kill']:
                t = pp.tile(list(din[name].shape), din[name].dtype, name=f'c_{name}')
                nc.sync.dma_start(t, din[name][:, :])
                cst[name] = t

            make_identity(nc, ident)
            nc.vector.memset(ones, 1.0)
            nc.vector.memset(onesb, 1.0)
            nc.vector.memset(ETm[64:65, :], 1.0)
            nc.vector.memset(hA, 0.0)
            nc.vector.memset(hA[64:65, :], 1.0)
            nc.vector.memset(hB[64:65, :], 1.0)

            # ================= PHASE A =================
            with tc.tile_pool(name='pA', bufs=6) as pa, \
                 tc.tile_pool(name='pSl', bufs=3) as psl, \
                 tc.tile_pool(name='pAp', bufs=8, space='PSUM') as pap:

                def conv_unit(u, margin, base):
                    xs1 = din['s1s'][u]
                    xs2 = din['s2s'][u]
                    bats = []
                    for b0 in range(0, 29, 8):
                        nb = min(8, 29 - b0)
                        s1x = psl.tile([128, nb * 500], bft, tag='slab1',
                                       padded_shape=[128, 4000], name='s1x')
                        s2x = psl.tile([81, nb * 500], bft, tag='slab2',
                                       padded_shape=[128, 4000], name='s2x')
                        bats.append((b0, nb, s1x, s2x))

                    def load(bat):
                        b0, nb, s1x, s2x = bat
                        # dst free = (ks, h); src (p, ks, h)
                        nc.sync.dma_start(
                            s1x[:, :],
                            _ap_raw(xs1[b0], [[500, 128], [64000, nb], [1, 500]]))
                        nc.sync.dma_start(
                            s2x[0:81, :],
                            _ap_raw(xs2[b0], [[500, 81], [40500, nb], [1, 500]]))

                    for bat in bats[:3]:
                        load(bat)
                    for bi, (b0, nb, s1x, s2x) in enumerate(bats):
                        for ks in range(nb):
                            k = b0 + ks
                            sl = slice(ks * 500, ks * 500 + 500)
                            s1 = s1x[:, sl]
                            s2 = s2x[0:81, sl]
                            psA = pap.tile([128, 500], f32, tag='ps',
                                           padded_shape=[128, 512])
                            psB = pap.tile([128, 500], f32, tag='ps',
                                           padded_shape=[128, 512])
                            nc.tensor.matmul(psA, cst['convA1'], s1, start=True, stop=False)
                            nc.tensor.matmul(psA, cst['convA2'], s2, start=False, stop=True)
                            nc.tensor.matmul(psB, cst['convB1'], s1, start=True, stop=False)
                            nc.tensor.matmul(psB, cst['convB2'], s2, start=False, stop=True)
                            pwb = pa.tile([128, 500], bft, tag='pwb')
                            nc.scalar.activation(pwb, psB, AF.Copy)
                            pw = pa.tile([128, 500], bft, tag='pw')
                            nc.vector.tensor_tensor(pw, psA, pwb, op=ALU.max)
                            m2 = pa.tile([128, 499], bft, tag='m2', padded_shape=[128, 512])
                            nc.vector.tensor_tensor(m2, pw[:, 0:499], pw[:, 1:500], op=ALU.max)
                            m4 = pa.tile([128, 497], bft, tag='m4', padded_shape=[128, 512])
                            nc.vector.tensor_tensor(m4, m2[:, 0:497], m2[:, 2:499], op=ALU.max)
                            pooled = pa.tile([128, 496], bft, tag='pool')
                            nc.vector.tensor_tensor(pooled, m4[:, 0:496], pw[:, 4:500], op=ALU.max)
                            wrow = base + 2 * k
                            if margin:
                                # only channels 59..63 feed warmup gi; store
                                # them in full: Pm[w, co''*496+s], co''=co-59
                                for jj in range(2):
                                    nc.sync.dma_start(
                                        _ap_mod(Pm2[wrow + jj:wrow + jj + 1, 0:1],
                                                [[496, 5], [1, 496]]),
                                        pooled[jj * 64 + 59:jj * 64 + 64, 0:496])
                            else:
                                nc.sync.dma_start(
                                    _ap_mod(P2g2[wrow:wrow + 2, 0:1],
                                            [[496, 64], [1, 496]]),
                                    pooled[0:128, 0:496])
                        if bi + 3 < len(bats):
                            load(bats[bi + 3])

                def lin_gi(br, b_loc, margin, base):
                    n = W if margin else 496
                    src = Pm2 if margin else P2g2
                    pl = pap.tile([64, 512], f32, tag='ps',
                                  padded_shape=[128, 512], name='pl')[:, 0:n]
                    for cp in range(64):
                        rhs = _ap_mod(src[base:base + 58, 0:1], [[64, n]],
                                      (1456 if margin else 0) + cp)
                        nc.tensor.matmul(
                            pl, cst['lin1w'][base:base + 58, cp * 64:(cp + 1) * 64],
                            rhs, start=(cp == 0), stop=False)
                    nc.tensor.matmul(pl, cst['lin1b'], onesb[0:1, 0:n],
                                     start=False, stop=True)
                    lo = pa.tile([64, 512], bft, tag='lo',
                                 padded_shape=[128, 512], name='lo')[:, 0:n]
                    nc.scalar.activation(lo, pl, AF.Copy)
                    gparts = [('giW_rz', 'giB_rz', gi_rz, 128),
                              ('giW_n', 'giB_n', gi_n, 64)]
                    for wname, bname, store, gp in gparts:
                        pg = pap.tile([gp, 512], f32, tag='ps',
                                      padded_shape=[128, 512],
                                      name='pg_' + wname)[:, 0:n]
                        nc.tensor.matmul(pg, cst[wname], lo, start=True, stop=False)
                        nc.tensor.matmul(pg, cst[bname], onesb[0:1, 0:n], start=False, stop=True)
                        if margin:
                            # psum col i = (c2, t8): dst col = t*128 + c
                            nc.scalar.activation(
                                _ap_mod(store[0:gp, 0:1], [[1, 2], [128, L]], br * GL),
                                pg, AF.Copy)
                        else:
                            # psum col s = (j62, t8): dst col = t*128 + (62*b_loc + j + 2)
                            off = br * GL + 62 * b_loc + 2
                            nc.scalar.activation(
                                _ap_mod(store[0:gp, 0:1], [[1, 62], [128, L]], off),
                                _ap_mod(pg, [[L, 62], [1, L]]), AF.Copy)

                # software pipeline: lin_gi(u) issues after conv_unit(u+1) so
                # the PE never head-of-line blocks on u's P2g store DMAs.
                unit_bases = [0, 0, 64, 64, 0, 64]
                units = [(0, 0, True), (0, 0, False), (0, 1, False),
                         (1, 0, True), (1, 0, False), (1, 1, False)]
                pending = []
                for u, (br, b_loc, margin) in enumerate(units):
                    base = unit_bases[u]
                    conv_unit(u, margin, base)
                    if pending:
                        lin_gi(*pending.pop())
                    pending.append((br, b_loc, margin, base))
                lin_gi(*pending.pop())

            # ================= PHASE B: GRU =================
            with tc.tile_pool(name='pB', bufs=2) as pb, \
                 tc.tile_pool(name='pBp', bufs=2, space='PSUM') as pbp:

                NC2 = 2 * CH

                def h_ap(t, p=65):
                    return _ap_mod(ETm[0:p, 0:1], [[1024, 2], [L, CH]], t)

                def gi_ap(store, p, t):
                    q, tp = divmod(t, L)
                    return _ap_mod(store[0:p, 0:1], [[GL, 2], [1, CH]],
                                   tp * 128 + 2 + q)

                killed = pp.tile([65, NC2], bft)
                nc.vector.memset(killed[64:65, :], 1.0)

                for i, t in enumerate(range(-W, L)):
                    if t < 0:
                        h_in = hA if i % 2 == 0 else hB
                        h_out_ap = (hB if i % 2 == 0 else hA)[0:64, :]
                    elif t == 0:
                        h_in = killed
                        h_out_ap = h_ap(0, 64)
                    else:
                        h_in = None
                        h_out_ap = h_ap(t, 64)

                    h_in_ap = h_in[0:65, :] if h_in is not None else h_ap(t - 1)
                    h_in64 = h_in[0:64, :] if h_in is not None else h_ap(t - 1, 64)

                    prz = pbp.tile([128, NC2], f32, tag='grz')
                    pn = pbp.tile([64, NC2], f32, tag='gn', padded_shape=[128, NC2])
                    nc.tensor.matmul(prz, cst['gruW_rz'], h_in_ap, start=True, stop=False)
                    nc.tensor.matmul(prz, ident, gi_ap(gi_rz, 128, t), start=False, stop=True)
                    nc.tensor.matmul(pn, cst['gruW_n'], h_in_ap, start=True, stop=True)
                    r = pb.tile([64, NC2], f32, tag='r', padded_shape=[128, NC2])
                    nc.scalar.activation(r, prz[0:64, :], AF.Sigmoid)
                    z = pb.tile([64, NC2], f32, tag='z', padded_shape=[128, NC2])
                    nc.scalar.activation(z, prz[64:128, :], AF.Sigmoid)
                    t2 = pb.tile([64, NC2], f32, tag='t2', padded_shape=[128, NC2])
                    nc.vector.tensor_mul(t2, r, pn)
                    npre = pb.tile([64, NC2], f32, tag='npre', padded_shape=[128, NC2])
                    nc.vector.tensor_add(npre, t2, gi_ap(gi_n, 64, t))
                    nt = pb.tile([64, NC2], f32, tag='nt', padded_shape=[128, NC2])
                    nc.scalar.activation(nt, npre, AF.Tanh)
                    dmn = pb.tile([64, NC2], f32, tag='dmn', padded_shape=[128, NC2])
                    nc.vector.scalar_tensor_tensor(dmn, nt, -1.0, h_in64,
                                                   op0=ALU.mult, op1=ALU.add)
                    e = pb.tile([64, NC2], f32, tag='e', padded_shape=[128, NC2])
                    nc.vector.tensor_mul(e, z, dmn)
                    nc.vector.tensor_add(h_out_ap, nt, e)
                    if t == -1:
                        last = hB if i % 2 == 0 else hA
                        nc.vector.tensor_mul(killed[0:64, :], last[0:64, :], cst['kill'])

            # ================= PHASE C =================
            with tc.tile_pool(name='pC', bufs=2) as pc_, \
                 tc.tile_pool(name='pCe', bufs=4) as pce, \
                 tc.tile_pool(name='pCp', bufs=2, space='PSUM') as pcp, \
                 tc.tile_pool(name='pCs', bufs=1, space='PSUM') as pcs:
                NB = [(0, 128), (128, 128), (256, 128), (384, 112)]
                for b_loc in range(2):
                    Es = ETm[0:64, b_loc * 496:b_loc * 496 + 496]
                    Ts = ETm[0:64, 1024 + b_loc * 496:1024 + b_loc * 496 + 496]
                    etiles, tmts = [], []
                    for (nb0, nbs) in NB:
                        psc = pcp.tile([128, 512], f32, tag='sc', name='psc')[0:nbs, 0:496]
                        nc.tensor.matmul(psc, Ts[:, nb0:nb0 + nbs], Es, start=True, stop=True)
                        nmx = pc_.tile([128, 1], f32, tag='nmx', name='nmx')[0:nbs, :]
                        nc.vector.tensor_reduce(nmx, psc, axis=mybir.AxisListType.X,
                                                op=ALU.max, negate=True)
                        et = pce.tile([128, 496], bft, tag='et', name='et')[0:nbs, :]
                        ssum = pc_.tile([128, 1], f32, tag='ssum', name='ssum')[0:nbs, :]
                        nc.scalar.activation(et, psc, AF.Exp, bias=nmx, accum_out=ssum)
                        rs = pc_.tile([128, 1], f32, tag='rs', name='rs')[0:nbs, :]
                        nc.vector.reciprocal(rs, ssum)
                        ptt = pcp.tile([128, 512], bft, tag='ptt', bufs=1, name='ptt')[0:nbs, 0:64]
                        nc.tensor.transpose(ptt, Ts[:, nb0:nb0 + nbs], ident[0:64, 0:64])
                        tmt = pce.tile([128, 64], bft, tag='tmt', name='tmt')[0:nbs, :]
                        nc.scalar.activation(tmt, ptt, AF.Copy, scale=rs)
                        etiles.append(et)
                        tmts.append(tmt)
                    ptp = pcs.tile([64, 512], f32, tag='tp', padded_shape=[128, 512], name='ptp')[:, 0:496]
                    for q, (nb0, nbs) in enumerate(NB):
                        nc.tensor.matmul(ptp, tmts[q], etiles[q],
                                         start=(q == 0), stop=(q == 3))
                    da = pc_.tile([64, 496], f32, tag='da', padded_shape=[128, 496])
                    nc.vector.tensor_sub(da, ptp, Es)
                    da2 = pc_.tile([64, 496], f32, tag='da2', padded_shape=[128, 496])
                    nc.scalar.activation(da2, da, AF.Abs)
                    patt = pcs.tile([1, 512], f32, tag='chain', padded_shape=[128, 512], bufs=2, name='patt')[:, 0:496]
                    nc.tensor.matmul(patt, cst['attw'], Es, start=True, stop=True)
                    anm = pc_.tile([1, 1], f32, tag='anm', padded_shape=[128, 1])
                    nc.vector.tensor_reduce(anm, patt, axis=mybir.AxisListType.X,
                                            op=ALU.max, negate=True)
                    ea = pc_.tile([1, 496], f32, tag='ea', padded_shape=[128, 496])
                    asum = pc_.tile([1, 1], f32, tag='asum', padded_shape=[128, 1])
                    nc.scalar.activation(ea, patt, AF.Exp, bias=anm, accum_out=asum)
                    ars = pc_.tile([1, 1], f32, tag='ars', padded_shape=[128, 1])
                    nc.vector.reciprocal(ars, asum)
                    pab = pcs.tile([64, 512], f32, tag='pab', padded_shape=[128, 512], name='pab')[:, 0:496]
                    nc.tensor.matmul(pab, ones[0:1, 0:64], ea, start=True, stop=True)
                    junk = pc_.tile([64, 496], f32, tag='junk', padded_shape=[128, 496])
                    nc.vector.tensor_mul(junk, da2, pab)
                    rep = pc_.tile([64, 1], f32, tag='rep', padded_shape=[128, 1])
                    nc.vector.tensor_reduce(rep, junk, axis=mybir.AxisListType.X,
                                            op=ALU.add)
                    prsb = pcs.tile([64, 1], f32, tag='chain', padded_shape=[128, 512], bufs=2)
                    nc.tensor.matmul(prsb, ones[0:1, 0:64], ars, start=True, stop=True)
                    rsb = pc_.tile([64, 1], f32, tag='rsb', padded_shape=[128, 1])
                    nc.scalar.activation(rsb, prsb, AF.Copy)
                    h1 = pc_.tile([64, 1], f32, tag='h1', padded_shape=[128, 1])
                    nc.scalar.activation(h1, rep, AF.Relu, scale=rsb)
                    ph2 = pcs.tile([128, 1], f32, tag='chain', padded_shape=[128, 512], bufs=2)
                    nc.tensor.matmul(ph2, cst['lin3T'], h1, start=True, stop=False)
                    nc.tensor.matmul(ph2, cst['lin3b'], ones[0:1, 0:1], start=False, stop=True)
                    h2 = pc_.tile([128, 1], f32, tag='h2')
                    nc.scalar.activation(h2, ph2, AF.Relu)
                    po = pcs.tile([2, 1], f32, tag='chain', padded_shape=[128, 512], bufs=2)
                    nc.tensor.matmul(po, cst['clsT'], h2, start=True, stop=False)
                    nc.tensor.matmul(po, cst['clsb'], ones[0:1, 0:1], start=False, stop=True)
                    osb = pc_.tile([2, 1], f32, tag='osb', padded_shape=[128, 1])
                    nc.scalar.activation(osb, po, AF.Copy)
                    nc.sync.dma_start(dout[b_loc:b_loc + 1, 0:2], osb)

    return nc


def kernel(**inputs):
    _install_bir_fix()
    inputs = {k: np.asarray(v) for k, v in inputs.items()}
    in_maps = _host_prep(inputs)
    if 'nc' not in _CACHE:
        _CACHE['nc'] = _build_program()
    nc = _CACHE['nc']
    from concourse.bass_utils import run_bass_kernel_spmd
    res = run_bass_kernel_spmd(nc, in_maps, core_ids=list(range(8)))
    out = np.zeros((16, 2), np.float32)
    for c in range(8):
        out[2 * c:2 * c + 2] = res.results[c]['outloc']
    return out

